# revision 21
# baseline (speedup 1.0000x reference)
"""Multi-head self-attention (B=4, T=2048, C=1024, H=16, D=64) on 8 NeuronCores.

Sharding: tensor-parallel over heads (Megatron): each core owns 2 heads.
Wq/Wk/Wv column-sharded, Wo row-sharded; host sums the 8 partial outputs.

Device layout is fully "transposed" (features on partitions, tokens on the
free dim) so that softmax runs over the PSUM free dim and the PV matmul needs
no attention-matrix transpose. The softmax denominator comes from a ones
column appended to V (M=65 stationary), landing in row 64 of the PV PSUM.

The attention inner loop is a flat software pipeline over (query-chunk, key-
tile) steps: QK + exp run at step g, PV lags LAG steps behind, and projection
/ output-projection matmuls of other batches are pulled in as fillers to keep
the PE saturated while ACT runs exp.
"""

import numpy as np

import concourse.bass as bass
import concourse.tile as tile
from concourse import bacc, mybir
from concourse.bass_utils import run_bass_kernel_spmd

B, T, C, H, D = 4, 2048, 1024, 16, 64
NCORES = 8
HPC = H // NCORES          # heads per core = 2
F = HPC * D                # per-core feature width = 128
TT = B * T                 # total tokens = 8192

FP32 = mybir.dt.float32
MM_DT = mybir.dt.bfloat16  # matmul compute dtype

TILE_K = 128               # contraction tile
TILE_N = 512               # moving free dim per matmul
NK_C = C // TILE_K         # 8 k-tiles over channels
NT4 = T // TILE_N          # 4 token chunks per batch
NJ = T // TILE_K           # 16 key tiles per batch
LAG = 4                    # pv lags qk/exp by this many steps


def build_kernel_body(tc):
    nc = tc.nc
    Exp = mybir.ActivationFunctionType.Exp

    xT = nc.dram_tensor("xT", [C, TT], MM_DT, kind="ExternalInput").ap()
    wq = nc.dram_tensor("wq", [C, F], MM_DT, kind="ExternalInput").ap()
    wk = nc.dram_tensor("wk", [C, F], MM_DT, kind="ExternalInput").ap()
    wv = nc.dram_tensor("wv", [C, F], MM_DT, kind="ExternalInput").ap()
    wo = nc.dram_tensor("wo", [F, C], MM_DT, kind="ExternalInput").ap()
    bqv = nc.dram_tensor("bq", [F], FP32, kind="ExternalInput").ap()
    bkv = nc.dram_tensor("bk", [F], FP32, kind="ExternalInput").ap()
    outT = nc.dram_tensor("outT", [C, TT], MM_DT, kind="ExternalOutput").ap()

    import contextlib
    ctx = contextlib.ExitStack()
    with ctx:
        consts = ctx.enter_context(tc.tile_pool(name="consts", bufs=1))
        xpool = ctx.enter_context(tc.tile_pool(name="xt", bufs=5))
        bigs = ctx.enter_context(tc.tile_pool(name="bigs", bufs=2))
        epool = ctx.enter_context(tc.tile_pool(name="expp", bufs=8))
        small = ctx.enter_context(tc.tile_pool(name="small", bufs=4))
        vstage = ctx.enter_context(tc.tile_pool(name="vstage", bufs=2))
        opool = ctx.enter_context(tc.tile_pool(name="ostage", bufs=2))
        ps_qk = ctx.enter_context(tc.tile_pool(name="ps_qk", bufs=2, space="PSUM"))
        ps_pv = ctx.enter_context(tc.tile_pool(name="ps_pv", bufs=1, space="PSUM"))
        ps_aux = ctx.enter_context(tc.tile_pool(name="ps_aux", bufs=2, space="PSUM"))

        # ---- constants; DMA order matters: the kernel starts with the K/V
        # projections of batch 0, so wk/wv (then batch 0's x chunks, issued by
        # gen_kv) must arrive first.
        wk_sb = consts.tile([TILE_K, C], MM_DT)
        nc.sync.dma_start(
            wk_sb[:].rearrange("p (k f) -> p k f", k=NK_C),
            wk.rearrange("(k p) f -> p k f", p=TILE_K))
        wv_sb = consts.tile([TILE_K, C], MM_DT)
        nc.sync.dma_start(
            wv_sb[:].rearrange("p (k f) -> p k f", k=NK_C),
            wv.rearrange("(k p) f -> p k f", p=TILE_K))
        ones32 = consts.tile([128, NJ * HPC], FP32)
        nc.gpsimd.memset(ones32[:], 1.0)
        # preload the exp table while DMAs are in flight (off critical path)
        warm = consts.tile([1, NJ * HPC], FP32)
        nc.scalar.activation(warm[:], ones32[0:1, :], Exp)

        xT_r = xT.rearrange("(k p) t -> p k t", p=TILE_K)
        outT_r = outT.rearrange("(o p) t -> p o t", p=128)

        tiles = {}  # per-batch SBUF tiles

        def load_x(b, t4):
            """Two batched DMAs (4 contraction tiles each) per token chunk so
            the first projection matmuls can start before the whole chunk
            lands."""
            xt = xpool.tile([TILE_K, NK_C, TILE_N], MM_DT, tag="xt")
            t0 = b * T
            tsl = slice(t0 + t4 * TILE_N, t0 + (t4 + 1) * TILE_N)
            h = NK_C // 2
            nc.sync.dma_start(xt[:, 0:1, :], xT_r[:, 0:1, tsl])
            nc.sync.dma_start(xt[:, 1:h, :], xT_r[:, 1:h, tsl])
            nc.sync.dma_start(xt[:, h:NK_C, :], xT_r[:, h:NK_C, tsl])
            return xt

        def alloc_tiles(b):
            qT = bigs.tile([F, T], MM_DT, tag="qT", name=f"qT{b}")
            kT = bigs.tile([F, T], MM_DT, tag="kT", name=f"kT{b}")
            v1 = bigs.tile([128, NJ * HPC, D + 1], MM_DT, tag="v1",
                           name=f"v1_{b}")
            nc.vector.tensor_copy(
                v1[:, :, D : D + 1],
                ones32[:].rearrange("p (a b) -> p a b", b=1),
            )
            tiles[b] = {"qT": qT, "kT": kT, "v1": v1, "xt": [None] * NT4}

        def gen_kv(b):
            """Generator: K and V projections for batch b; yields per PE op."""
            alloc_tiles(b)
            tl = tiles[b]
            for t4 in range(NT4):
                tl["xt"][t4] = load_x(b, t4)
            for t4 in range(NT4):
                xt = tl["xt"][t4]
                for which, w_sb in (("k", wk_sb), ("v", wv_sb)):
                    if which == "k":
                        acc = ps_aux.tile([128, TILE_N], FP32, tag="aux")
                        for kk in range(NK_C):
                            nc.tensor.matmul(
                                acc[:], w_sb[:, kk * F : (kk + 1) * F],
                                xt[:, kk, :],
                                start=(kk == 0), stop=(kk == NK_C - 1),
                            )
                            yield
                        nc.vector.tensor_scalar_add(
                            tl["kT"][:, t4 * TILE_N : (t4 + 1) * TILE_N],
                            acc[:], bk_sb[:])
                    else:
                        # V computed directly in [token, feature] orientation
                        # (stationary = x chunk, moving = Wv): lands exactly
                        # as the PV stationary needs it -- no PE transposes.
                        for tt in range(TILE_N // 128):
                            j_idx = t4 * (TILE_N // 128) + tt
                            csl = slice(tt * 128, (tt + 1) * 128)
                            vacc = ps_aux.tile([128, TILE_N], FP32,
                                               tag="aux", name="vacc")
                            for kk in range(NK_C):
                                nc.tensor.matmul(
                                    vacc[:, 0:128], xt[:, kk, csl],
                                    w_sb[:, kk * F : (kk + 1) * F],
                                    start=(kk == 0), stop=(kk == NK_C - 1),
                                )
                                if kk % 4 == 3:
                                    yield
                            # both heads' v columns in one strided copy
                            nc.vector.tensor_copy(
                                tl["v1"][:, j_idx * HPC : (j_idx + 1) * HPC, 0:D],
                                vacc[:, 0:128].rearrange("p (h d) -> p h d", h=HPC),
                            )

        def gen_q(b, t4s):
            """Generator: Q projection chunks for batch b; yields per PE op."""
            tl = tiles[b]
            for t4 in t4s:
                xt = tl["xt"][t4]
                acc = ps_aux.tile([128, TILE_N], FP32, tag="aux")
                for kk in range(NK_C):
                    nc.tensor.matmul(
                        acc[:], wq_sb[:, kk * F : (kk + 1) * F],
                        xt[:, kk, :],
                        start=(kk == 0), stop=(kk == NK_C - 1),
                    )
                    yield
                nc.vector.tensor_scalar_add(
                    tl["qT"][:, t4 * TILE_N : (t4 + 1) * TILE_N],
                    acc[:], bq_sb[:])
                tl["xt"][t4] = None  # release the x chunk

        def gen_wo_t4(b, t4):
            """Generator: output projection chunk; yields per PE op."""
            t0 = b * T
            ctxT = tiles[b]["ctxT"]
            osb = opool.tile([128, C // 128, TILE_N], MM_DT, tag="osb")
            tsl = slice(t0 + t4 * TILE_N, t0 + (t4 + 1) * TILE_N)
            for o in range(C // 128):
                po = ps_aux.tile([128, TILE_N], FP32, tag="aux")
                nc.tensor.matmul(
                    po[:], wo_sb[:, o * 128 : (o + 1) * 128],
                    ctxT[:, t4 * TILE_N : (t4 + 1) * TILE_N],
                    start=True, stop=True,
                )
                nc.vector.tensor_copy(osb[:, o, :], po[:])
                if o % 4 == 3:
                    nc.sync.dma_start(
                        outT_r[:, o - 3 : o + 1, tsl], osb[:, o - 3 : o + 1, :])
                yield

        fillers = []
        alive = set()

        def add_filler(gen):
            fillers.append(gen)
            alive.add(gen)
            return gen

        def pull(budget):
            while budget > 0 and fillers:
                try:
                    next(fillers[0])
                    budget -= 1
                except StopIteration:
                    alive.discard(fillers.pop(0))

        # small/early consts before the bulk x traffic: q(0) is on the
        # serial prologue critical path, so wq/bq/bk must not queue behind
        # 4.5MB of x loads.
        wq_sb = consts.tile([TILE_K, C], MM_DT)
        nc.sync.dma_start(
            wq_sb[:].rearrange("p (k f) -> p k f", k=NK_C),
            wq.rearrange("(k p) f -> p k f", p=TILE_K))
        bk_sb = consts.tile([F, 1], FP32)
        nc.sync.dma_start(bk_sb[:], bkv.rearrange("(p one) -> p one", one=1))
        bq_sb = consts.tile([F, 1], FP32)
        nc.sync.dma_start(bq_sb[:], bqv.rearrange("(p one) -> p one", one=1))

        # prologue: only what batch 0's first attention chunk needs.
        kv0 = gen_kv(0)
        next(kv0)  # issue batch 0's x DMAs + first matmul right away

        wo_sb = consts.tile([F, C], MM_DT)
        nc.sync.dma_start(wo_sb[:], wo)

        deferred = {}
        holdback = []
        q3b = [None]
        for _ in kv0:
            pass
        for _ in gen_q(0, [0]):
            pass
        proj_gens = {0: [add_filler(gen_q(0, [1, 2, 3]))]}

        for b in range(B):
            tl = tiles[b]
            ctxT = bigs.tile([F, T], MM_DT, tag="ctxT", name=f"ctxT{b}")
            tl["ctxT"] = ctxT
            qT, kT, v1 = tl["qT"], tl["kT"], tl["v1"]

            # Emission order IS dependency order in the tile framework: batch
            # b's projections (and anything queued before them) must be fully
            # emitted before b's attention reads qT/kT/v1.
            while any(g in alive for g in proj_gens.get(b, ())):
                pull(1)

            if b + 1 < B:
                if b + 1 == 3:
                    # batch 3's loop is filler-starved: push its last two Q
                    # chunks into it (guarded mid-loop, needed from g=32)
                    proj_gens[3] = [add_filler(gen_kv(3)),
                                    add_filler(gen_q(3, [0, 1]))]
                    q3b[0] = gen_q(3, [2, 3])
                    deferred.setdefault(3, []).append(q3b[0])
                else:
                    proj_gens[b + 1] = [add_filler(gen_kv(b + 1)),
                                        add_filler(gen_q(b + 1, [0, 1, 2, 3]))]
            for g_wo in reversed(deferred.pop(b, [])):
                fillers.insert(0, g_wo)
                alive.add(g_wo)

            NG = NT4 * NJ  # 64 flat steps: g -> (i4 = g//NJ, j = g%NJ)
            expts = [None] * NG
            pvs = [None] * NT4

            def pv_step(g):
                i4p, jp = divmod(g, NJ)
                if jp == 0:
                    pvs[i4p] = ps_pv.tile([128, HPC, TILE_N], FP32, tag="pv",
                                          name=f"pv{i4p}")
                pv = pvs[i4p]
                e = expts[g]
                nc.tensor.matmul(
                    pv[0 : D + 1, 0, :], v1[:, jp * HPC + 0, :],
                    e[:, 0:TILE_N], start=(jp == 0), stop=(jp == NJ - 1))
                nc.tensor.matmul(
                    pv[0 : D + 1, 1, :], v1[:, jp * HPC + 1, :],
                    e[:, TILE_N : 2 * TILE_N], start=(jp == 0),
                    stop=(jp == NJ - 1))
                expts[g] = None
                if jp == NJ - 1:
                    finish_i4(i4p)

            def finish_i4(i4p):
                """One fast PSUM->SBUF copy releases the pv banks; the
                normalization then runs from SBUF off the PE critical path."""
                isl = slice(i4p * TILE_N, (i4p + 1) * TILE_N)
                pv = pvs[i4p]
                cst = small.tile([D + 1, HPC * TILE_N], FP32, tag="cst")
                nc.vector.tensor_copy(
                    cst[:], pv[:D + 1, :, :].rearrange("p h n -> p (h n)"))
                dn = small.tile([1, HPC * TILE_N], FP32, tag="dn")
                nc.vector.tensor_copy(dn[:], cst[D : D + 1, :])
                rd = small.tile([1, HPC * TILE_N], FP32, tag="rd")
                # note: reciprocal_approx_fast needs a partition-0 input AP
                nc.vector.reciprocal_approx_fast(rd[:], dn[:])
                bc = small.tile([D, HPC * TILE_N], FP32, tag="bc")
                nc.gpsimd.partition_broadcast(bc[:], rd[:])
                for h in range(HPC):
                    nc.vector.tensor_mul(
                        ctxT[h * D : (h + 1) * D, isl],
                        cst[0:D, h * TILE_N : (h + 1) * TILE_N],
                        bc[:, h * TILE_N : (h + 1) * TILE_N])
                pvs[i4p] = None
                g_wo = gen_wo_t4(b, i4p)
                if b <= 1 and i4p >= 2:
                    # feed batch b+2's early slots; front-inserted there so
                    # it is emitted before ctxT(b)'s buffer is rewritten
                    deferred.setdefault(b + 2, []).append(g_wo)
                elif b == 2 and i4p >= 2:
                    deferred.setdefault(3, []).append(g_wo)
                else:
                    add_filler(g_wo)

            HOLD = 2  # extra pv delay entering a new i4 (pv-bank WAR slack)
            pv_done = 0

            def pv_eligible(p, g):
                lag = LAG + (HOLD if (p % NJ == 0 and p > 0) else 0)
                return g >= p + lag

            for g in range(NG):
                i4, j = divmod(g, NJ)
                if b == 3 and g == NJ:
                    # qT(3) chunks 2/3 must be emitted before g=32 reads them
                    while q3b[0] in alive:
                        pull(1)
                isl = slice(i4 * TILE_N, (i4 + 1) * TILE_N)
                jsl = slice(j * TILE_K, (j + 1) * TILE_K)
                qk = ps_qk.tile([128, 2 * TILE_N], FP32, tag="qk")
                # heads in distinct PE row-groups -> run concurrently
                nc.tensor.matmul(qk[:, 0:TILE_N], kT[0:D, jsl],
                                 qT[0:D, isl], start=True, stop=True)
                nc.tensor.matmul(qk[:, TILE_N : 2 * TILE_N],
                                 kT[D : 2 * D, jsl], qT[D : 2 * D, isl],
                                 start=True, stop=True)
                expt = epool.tile([128, 2 * TILE_N], MM_DT, tag="expt")
                nc.scalar.activation(expt[:], qk[:], Exp)
                expts[g] = expt
                # keep the in-order PE stream fed while ACT runs exp
                pull(2 if (b < 3 or g < NJ) else 1)
                nsteps = 0
                while (pv_done < NG and nsteps < 2
                       and pv_eligible(pv_done, g)):
                    pv_step(pv_done)
                    pv_done += 1
                    nsteps += 1
            while pv_done < NG:
                pv_step(pv_done)
                pv_done += 1
                pull(1)

        # the held-back wo chunk overlaps the last normalization chain
        for g_wo in holdback:
            add_filler(g_wo)
        # drain remaining fillers (last batch's final wo chunks)
        pull(10 ** 9)


_CACHE = {}


def _get_nc():
    if "nc" not in _CACHE:
        nc = bacc.Bacc("TRN2", target_bir_lowering=False, debug=False,
                       num_devices=NCORES)
        with tile.TileContext(nc) as tc:
            build_kernel_body(tc)
        nc.compile()
        _CACHE["nc"] = nc
    return _CACHE["nc"]


def host_prep(x, Wq, bq, Wk, bk, Wv, bv, Wo, bo):
    import ml_dtypes
    bf16 = ml_dtypes.bfloat16
    x = np.asarray(x, np.float32)
    xT = np.ascontiguousarray(x.reshape(TT, C).T.astype(bf16))
    scale = np.float32(1.0 / np.sqrt(D))
    in_maps = []
    for c in range(NCORES):
        fsl = slice(c * F, (c + 1) * F)
        in_maps.append({
            "xT": xT,
            "wq": np.ascontiguousarray(
                (np.asarray(Wq, np.float32)[:, fsl] * scale).astype(bf16)),
            "wk": np.ascontiguousarray(np.asarray(Wk, np.float32)[:, fsl].astype(bf16)),
            "wv": np.ascontiguousarray(np.asarray(Wv, np.float32)[:, fsl].astype(bf16)),
            "wo": np.ascontiguousarray(np.asarray(Wo, np.float32)[fsl, :].astype(bf16)),
            "bq": np.ascontiguousarray(np.asarray(bq, np.float32)[fsl] * scale),
            "bk": np.ascontiguousarray(np.asarray(bk, np.float32)[fsl]),
        })
    return in_maps


def host_gather(results, Wo, bo, bv):
    total = np.zeros((C, TT), np.float64)
    for c in range(NCORES):
        total += results[c]["outT"].astype(np.float64)
    out = total.T.astype(np.float32)
    out = out + (np.asarray(bo, np.float32)
                 + np.asarray(bv, np.float32) @ np.asarray(Wo, np.float32))
    return out.reshape(B, T, C)


def _install_profile_hook():
    """Make trace=True work under axon when antenv.axon_hooks is absent."""
    import sys
    import types

    try:
        import antenv.axon_hooks  # noqa: F401
        return
    except ImportError:
        pass
    import antenv
    from trn_agent_boot.trn_boot import _ntff_profile_via_ctypes

    mod = types.ModuleType("antenv.axon_hooks")
    holder = [None]
    mod.set_axon_ntff_profile_hook = lambda h: holder.__setitem__(0, h)
    mod.get_axon_ntff_profile_hook = lambda: holder[0]
    sys.modules["antenv.axon_hooks"] = mod
    antenv.axon_hooks = mod
    mod.set_axon_ntff_profile_hook(
        _ntff_profile_via_ctypes("/opt/axon/libaxon_pjrt.so")
    )
    # artifact upload needs internal storage; keep profiles local
    import concourse.bass_utils as bu
    bu.upload_artifacts = lambda tmpdir: f"local:{tmpdir}"


def kernel(x, Wq, bq, Wk, bk, Wv, bv, Wo, bo, _trace=False):
    if _trace:
        _install_profile_hook()
    nc = _get_nc()
    in_maps = host_prep(x, Wq, bq, Wk, bk, Wv, bv, Wo, bo)
    res = run_bass_kernel_spmd(nc, in_maps, core_ids=list(range(NCORES)),
                               trace=_trace)
    _CACHE["last_result"] = res
    return host_gather(res.results, Wo, bo, bv)
